# revision 1
# baseline (speedup 1.0000x reference)
"""Trainium2 Bass kernel for a batched-ensemble MLP (nn_BMLP_773094113632).

Network per ensemble member e (64 members):
    u = silu(x @ w0 + b0); u = silu(u @ w1 + b1); u = silu(u @ w2 + b2)
    y = u @ wl + bl
Shapes: x [64, 4096, 16], hidden 256, out 1.

Strategy (v2): shard the 64 members across 8 NeuronCores (8 per core).
All matmuls run in bf16 (fp32 PSUM accumulate).  Activations live in SBUF
as [hidden(part), points(free)] bf16 tiles.  Layer 0 runs K=16 matmuls with
the bias applied through the scalar-engine activation bias port; layers 1/2
chain K=128 matmuls accumulated over 2 k-tiles.  The SiLU+bias PSUM->SBUF
evacuation on the scalar engine is the kernel bottleneck (~1 elem/lane/cyc),
so the schedule keeps the scalar engine saturated: 2 ping-pong PSUM tiles
of [128, 2048] rotate between PE fill and ACT drain.  The final [256 -> 1]
layer is column-tiled: 4 members' M=1 matmuls run concurrently in the four
32-column PE groups (outputs at partitions 0/32/64/96), interleaved into the
next member's layer-0/1 stream; its bias is added by the vector engine and
the rows DMA out with a partition-strided access pattern.
"""

import sys

sys.path.insert(0, "/opt/trn_rl_repo")

import numpy as np

import concourse.tile as tile
from concourse import bacc, mybir

F32 = mybir.dt.float32
F32R = mybir.dt.float32r
BF16 = mybir.dt.bfloat16
AFT = mybir.ActivationFunctionType

E = 64  # ensemble members
NPTS = 4096
INDIM = 16
HID = 256
N_CORES = 8
EPC = E // N_CORES  # members per core
MMQ = 512  # matmul N chunk (one fp32 PSUM bank)
GRP = 2048  # points group width (4 PSUM banks)
NG = NPTS // GRP  # groups
GQ = GRP // MMQ  # matmul chunks per group
BLK = 4  # members per final-layer column-tiled block


def build(
    reps: int = 1,
    dtype=F32R,
    hw_loop: bool = False,
    passes: int = 1,
    **_unused,
):
    nc = bacc.Bacc("TRN2", target_bir_lowering=False, debug=False)

    xt_d = nc.dram_tensor("xt", [EPC, INDIM, NPTS], BF16, kind="ExternalInput").ap()
    w0_d = nc.dram_tensor("w0p", [EPC, INDIM, HID], BF16, kind="ExternalInput").ap()
    w1_d = nc.dram_tensor("w1p", [EPC, 128, 512], BF16, kind="ExternalInput").ap()
    w2_d = nc.dram_tensor("w2p", [EPC, 128, 512], BF16, kind="ExternalInput").ap()
    wl_d = nc.dram_tensor("wlp", [128, 2 * EPC], BF16, kind="ExternalInput").ap()
    bias_d = nc.dram_tensor("biasp", [128, 6 * EPC], F32, kind="ExternalInput").ap()
    bl_d = nc.dram_tensor("blp", [128, EPC // BLK], F32, kind="ExternalInput").ap()
    y_d = nc.dram_tensor("y", [EPC, NPTS], F32, kind="ExternalOutput").ap()

    with tile.TileContext(nc) as tc:
        with (
            tc.tile_pool(name="const", bufs=1) as const_pool,
            tc.tile_pool(name="w", bufs=2) as w_pool,
            tc.tile_pool(name="xt", bufs=4) as xt_pool,
            tc.tile_pool(name="u01", bufs=9) as u01_pool,
            tc.tile_pool(name="u2", bufs=BLK * 2 * NG + 2) as u2_pool,
            tc.tile_pool(name="y", bufs=2) as y_pool,
            tc.tile_pool(name="ps", bufs=2, space="PSUM") as ps_pool,
        ):
            wl_sb = const_pool.tile([128, 2 * EPC], BF16, tag="wl")
            nc.sync.dma_start(wl_sb[:], wl_d)
            bias_sb = const_pool.tile([128, 6 * EPC], F32, tag="bias")
            nc.sync.dma_start(bias_sb[:], bias_d)
            bl_sb = const_pool.tile([128, EPC // BLK], F32, tag="bl")
            nc.sync.dma_start(bl_sb[:], bl_d)

            def bias_ap(e, ly, mt):
                c = e * 6 + ly * 2 + mt
                return bias_sb[:, c : c + 1]


            def emit_l0(e, w0_sb, xt_sb, u, g, mt):
                ps = ps_pool.tile([128, GRP], F32, tag="ps")
                for q in range(GQ):
                    nc.tensor.matmul(
                        ps[:, q * MMQ : (q + 1) * MMQ],
                        w0_sb[0:INDIM, mt * 128 : (mt + 1) * 128],
                        xt_sb[0:INDIM, q * MMQ : (q + 1) * MMQ],
                        start=True,
                        stop=True,
                    )
                u0 = u01_pool.tile([128, GRP], BF16, tag="u")
                nc.scalar.activation(u0[:], ps[:], AFT.Silu, bias=bias_ap(e, 0, mt))
                u[0][mt][g] = u0

            def emit_l12(e, ly, w_sb, u, g, mt):
                ps = ps_pool.tile([128, GRP], F32, tag="ps")
                for kt in range(2):
                    for q in range(GQ):
                        nc.tensor.matmul(
                            ps[:, q * MMQ : (q + 1) * MMQ],
                            w_sb[:, kt * HID + mt * 128 : kt * HID + (mt + 1) * 128],
                            u[ly - 1][kt][g][:, q * MMQ : (q + 1) * MMQ],
                            start=(kt == 0),
                            stop=(kt == 1),
                        )
                pool = u2_pool if ly == 2 else u01_pool
                ul = pool.tile([128, GRP], BF16, tag="u2" if ly == 2 else "u")
                nc.scalar.activation(ul[:], ps[:], AFT.Silu, bias=bias_ap(e, ly, mt))
                u[ly][mt][g] = ul

            def emit_l3(bi, u2s, g):
                """Column-tiled final layer: members 4*bi..4*bi+3, one group.

                u2s[j] = member (4*bi+j)'s u[2] tiles, indexed [kt][g].
                Output rows land at PSUM partitions 32*j.
                """
                ps = ps_pool.tile([128, GRP], F32, tag="ps")
                for q in range(GQ):
                    for kt in range(2):
                        for j in range(BLK):
                            col = (bi * BLK + j) * 2 + kt
                            nc.tensor.matmul(
                                ps[32 * j : 32 * j + 1, q * MMQ : (q + 1) * MMQ],
                                wl_sb[:, col : col + 1],
                                u2s[j][kt][g][:, q * MMQ : (q + 1) * MMQ],
                                start=(kt == 0),
                                stop=(kt == 1),
                                tile_position=(0, 32 * j),
                            )
                y_sb = y_pool.tile([128, GRP], F32, tag="y")
                nc.vector.tensor_scalar_add(y_sb[:], ps[:], bl_sb[:, bi : bi + 1])
                nc.sync.dma_start(
                    y_d[bi * BLK : (bi + 1) * BLK, g * GRP : (g + 1) * GRP],
                    y_sb[0 : 32 * BLK : 32, :],
                )

            def one_pass():
                # Pending column-tiled final-layer work is interleaved into
                # the next member's layer-0/1 stream to keep ACT fed.
                pending = None  # (block index, [u2 tiles per member])
                block_u2 = []
                for e in range(EPC):
                    w0_sb = w_pool.tile([128, HID], BF16, tag="w0")
                    nc.sync.dma_start(w0_sb[0:INDIM, :], w0_d[e])
                    w1_sb = w_pool.tile([128, 512], BF16, tag="w12", bufs=4)
                    nc.sync.dma_start(w1_sb[:], w1_d[e])
                    w2_sb = w_pool.tile([128, 512], BF16, tag="w12", bufs=4)
                    nc.sync.dma_start(w2_sb[:], w2_d[e])

                    # u[layer][mt][group] -> SBUF tile [128, GRP]
                    u = [[[None] * NG, [None] * NG] for _ in range(3)]

                    for g in range(NG):
                        xt_sb = xt_pool.tile([128, GRP], BF16, tag="xt")
                        nc.sync.dma_start(
                            xt_sb[0:INDIM, :], xt_d[e][:, g * GRP : (g + 1) * GRP]
                        )
                        emit_l0(e, w0_sb, xt_sb, u, g, 0)
                        emit_l0(e, w0_sb, xt_sb, u, g, 1)
                        if pending is not None:
                            emit_l3(pending[0], pending[1], g)
                            if g == NG - 1:
                                pending = None
                    for g in range(NG):
                        emit_l12(e, 1, w1_sb, u, g, 0)
                        emit_l12(e, 1, w1_sb, u, g, 1)
                    for g in range(NG):
                        emit_l12(e, 2, w2_sb, u, g, 0)
                        emit_l12(e, 2, w2_sb, u, g, 1)

                    block_u2.append(u[2])
                    if len(block_u2) == BLK:
                        pending = (e // BLK, block_u2)
                        block_u2 = []

                for g in range(NG):
                    emit_l3(pending[0], pending[1], g)

            if hw_loop:
                hints = (
                    (
                        mybir.EngineType.PE,
                        mybir.EngineType.Activation,
                        mybir.EngineType.SP,
                        mybir.EngineType.DVE,
                    )
                    if hw_loop == "hints"
                    else ()
                )
                with tc.For_i(
                    0,
                    reps,
                    1,
                    staggered_reset=hw_loop == "staggered",
                    hint_engines=hints,
                ):
                    for _ in range(passes):
                        one_pass()
            else:
                for _ in range(reps):
                    one_pass()

    nc.compile()
    return nc


def pack_inputs(x, w0, b0, w1, b1, w2, b2, wl, bl):
    """Split the full-ensemble inputs into 8 per-core input maps (bf16)."""
    import ml_dtypes

    bf = ml_dtypes.bfloat16
    f = np.float32
    in_maps = []
    for c in range(N_CORES):
        sl = slice(c * EPC, (c + 1) * EPC)
        xt = np.ascontiguousarray(
            np.asarray(x[sl]).transpose(0, 2, 1), dtype=bf
        )  # [e, 16, npts]
        w0p = np.ascontiguousarray(w0[sl], dtype=bf)  # [e, 16, 256]
        # [e, 256, 256] -> [e, 128(p), (kt*2+mt)*128+c] with kt = input k-tile
        w1p = np.ascontiguousarray(
            np.asarray(w1[sl])
            .reshape(EPC, 2, 128, 2, 128)
            .transpose(0, 2, 1, 3, 4)
            .reshape(EPC, 128, 512),
            dtype=bf,
        )
        w2p = np.ascontiguousarray(
            np.asarray(w2[sl])
            .reshape(EPC, 2, 128, 2, 128)
            .transpose(0, 2, 1, 3, 4)
            .reshape(EPC, 128, 512),
            dtype=bf,
        )
        # [e, 256, 1] -> [128(p), e*2+kt]
        wlp = np.ascontiguousarray(
            np.asarray(wl[sl]).reshape(EPC, 2, 128).transpose(2, 0, 1).reshape(128, 2 * EPC),
            dtype=bf,
        )
        # [128(p), e*6 + ly*2 + mt]
        biasp = np.zeros((128, 6 * EPC), f)
        for e in range(EPC):
            for ly, b in enumerate((b0, b1, b2)):
                for mt in range(2):
                    biasp[:, e * 6 + ly * 2 + mt] = np.asarray(
                        b[c * EPC + e, 0, mt * 128 : (mt + 1) * 128], dtype=f
                    )
        blp = np.zeros((128, EPC // BLK), f)
        for bi in range(EPC // BLK):
            for j in range(BLK):
                blp[32 * j, bi] = np.float32(bl[c * EPC + bi * BLK + j, 0, 0])
        in_maps.append(
            {
                "xt": xt,
                "w0p": w0p,
                "w1p": w1p,
                "w2p": w2p,
                "wlp": wlp,
                "biasp": biasp,
                "blp": blp,
            }
        )
    return in_maps


def make_runner(nc):
    """Compile nc once into a persistent 8-core jitted callable."""
    import jax
    from jax.experimental.shard_map import shard_map
    from jax.sharding import Mesh, PartitionSpec

    from concourse import bass2jax, mybir as _mybir

    bass2jax.install_neuronx_cc_hook()

    partition_name = nc.partition_id_tensor.name if nc.partition_id_tensor else None
    in_names, out_names, out_avals, zero_outs = [], [], [], []
    for alloc in nc.m.functions[0].allocations:
        if not isinstance(alloc, _mybir.MemoryLocationSet):
            continue
        name = alloc.memorylocations[0].name
        if alloc.kind == "ExternalInput":
            if name != partition_name:
                in_names.append(name)
        elif alloc.kind == "ExternalOutput":
            out_names.append(name)
            shape = tuple(alloc.tensor_shape)
            dt = _mybir.dt.np(alloc.dtype)
            out_avals.append(jax.core.ShapedArray(shape, dt))
            zero_outs.append(np.zeros(shape, dt))
    n_params = len(in_names)
    n_outs = len(out_names)
    all_names = in_names + out_names
    if partition_name is not None:
        all_names = all_names + [partition_name]
    donate = tuple(range(n_params, n_params + n_outs))

    def _body(*args):
        operands = list(args)
        if partition_name is not None:
            operands.append(bass2jax.partition_id_tensor())
        outs = bass2jax._bass_exec_p.bind(
            *operands,
            out_avals=tuple(out_avals),
            in_names=tuple(all_names),
            out_names=tuple(out_names),
            lowering_input_output_aliases=(),
            sim_require_finite=True,
            sim_require_nnan=True,
            nc=nc,
        )
        return tuple(outs)

    devices = jax.devices()[:N_CORES]
    mesh = Mesh(np.asarray(devices), ("core",))
    sharded = jax.jit(
        shard_map(
            _body,
            mesh=mesh,
            in_specs=(PartitionSpec("core"),) * (n_params + n_outs),
            out_specs=(PartitionSpec("core"),) * n_outs,
            check_rep=False,
        ),
        donate_argnums=donate,
        keep_unused=True,
    )

    state = {}

    def run(in_maps, cache_inputs=False):
        if not cache_inputs or "dev_in" not in state:
            import jax

            concat_in = [
                np.concatenate([np.asarray(m[name]) for m in in_maps], axis=0)
                for name in in_names
            ]
            state["dev_in"] = [jax.device_put(a) for a in concat_in]
            for a in state["dev_in"]:
                a.block_until_ready()
        concat_zeros = [
            np.zeros((N_CORES * z.shape[0], *z.shape[1:]), z.dtype) for z in zero_outs
        ]
        out_arrs = sharded(*state["dev_in"], *concat_zeros)
        out_arrs = [np.asarray(o) for o in out_arrs]
        return [
            {
                name: out_arrs[i].reshape(N_CORES, *out_avals[i].shape)[c]
                for i, name in enumerate(out_names)
            }
            for c in range(N_CORES)
        ]

    return run


_RUNNER_CACHE = {}


def _get_runner(reps=1, hw_loop=False, passes=1, **bkw):
    key = (reps, hw_loop, passes, tuple(sorted(bkw.items())))
    if key not in _RUNNER_CACHE:
        _RUNNER_CACHE[key] = make_runner(
            build(reps, hw_loop=hw_loop, passes=passes, **bkw)
        )
    return _RUNNER_CACHE[key]


def run(in_maps, reps=1, hw_loop=False, cache_inputs=False, passes=1, **bkw):
    return _get_runner(reps, hw_loop, passes, **bkw)(in_maps, cache_inputs=cache_inputs)


def kernel(x, w0, b0, w1, b1, w2, b2, wl, bl):
    in_maps = pack_inputs(x, w0, b0, w1, b1, w2, b2, wl, bl)
    results = run(in_maps)
    y = np.concatenate([results[c]["y"] for c in range(N_CORES)], axis=0)
    return y.reshape(E, NPTS, 1).astype(np.float32)


if __name__ == "__main__":
    rng = np.random.default_rng(0)
    ins = {
        "x": rng.standard_normal((E, NPTS, INDIM), dtype=np.float32),
        "w0": rng.standard_normal((E, INDIM, HID), dtype=np.float32) * 0.25,
        "b0": rng.standard_normal((E, 1, HID), dtype=np.float32) * 0.25,
        "w1": rng.standard_normal((E, HID, HID), dtype=np.float32) * 0.06,
        "b1": rng.standard_normal((E, 1, HID), dtype=np.float32) * 0.06,
        "w2": rng.standard_normal((E, HID, HID), dtype=np.float32) * 0.06,
        "b2": rng.standard_normal((E, 1, HID), dtype=np.float32) * 0.06,
        "wl": rng.standard_normal((E, HID, 1), dtype=np.float32) * 0.06,
        "bl": rng.standard_normal((E, 1, 1), dtype=np.float32) * 0.06,
    }
    out = kernel(**ins)

    def silu(v):
        return v / (1.0 + np.exp(-v))

    u = silu(ins["x"] @ ins["w0"] + ins["b0"])
    u = silu(u @ ins["w1"] + ins["b1"])
    u = silu(u @ ins["w2"] + ins["b2"])
    ref = u @ ins["wl"] + ins["bl"]
    err = np.abs(out - ref).max() / np.abs(ref).max()
    print("self-test rel err:", err)

